# revision 23
# baseline (speedup 1.0000x reference)
"""Bass/Trainium2 kernel for EnhancedGNNCap message passing (8 NeuronCores).

Strategy (node-sharded, edge-sorted, merged-lhsT edge phase, host-gathered x_j):
  - Sort edges by dst on host; shard nodes (and incoming edges) across 8
    cores; windows of 120 dst nodes; pad each window's edges to 128-edge
    tiles.
  - Host packs three [128, T*128] fp16 streams:
      st_m: per tile lhsT = [S_T(120); ea^T(7); ones(1)]  (merged gather)
      xjt:  per tile lhsT = x[src_e]^T                    (host-gathered)
      s_oh: per tile rhs  = one-hot S [e, node]           (scatter)
  - Phase 0 (device): P_i composite tiles [P_i_win; W1e; b1] (f32 matmul,
    fp16 store).
  - Edge phase per tile: h_ps = st_m.T @ comp + xjt.T @ W1j (PSUM
    accumulate), ReLU (4 tiles batched) -> fp16, scatter-accumulate
    A_T += h.T @ S.
  - Window close: aggr_T = W2.T @ A_T + b2 (x) deg  (f32).
  - Node phase: GRU + gate + LayerNorm in [ch, node] orientation,
    transpose, write out.
All per-core differences are carried in input data; one SPMD program.
"""

import os
import sys
import types

sys.path.insert(0, "/opt/trn_rl_repo")

import numpy as np


def _install_ntff_hook():
    """Register the axon NTFF profiling hook if the image lacks antenv.axon_hooks."""
    try:
        import antenv
        try:
            import antenv.axon_hooks  # noqa: F401
            return
        except ImportError:
            pass
        m = types.ModuleType("antenv.axon_hooks")
        m._hook = None
        m.set_axon_ntff_profile_hook = lambda h: setattr(m, "_hook", h)
        m.get_axon_ntff_profile_hook = lambda: m._hook
        sys.modules["antenv.axon_hooks"] = m
        antenv.axon_hooks = m
        from trn_agent_boot.trn_boot import _ntff_profile_via_ctypes
        m.set_axon_ntff_profile_hook(_ntff_profile_via_ctypes("/opt/axon/libaxon_pjrt.so"))
    except Exception:
        pass


_install_ntff_hook()

import concourse.bass as bass  # noqa: E402
import concourse.bacc as bacc  # noqa: E402
import concourse.mybir as mybir  # noqa: E402
import concourse.tile as tile  # noqa: E402
from concourse.masks import make_identity  # noqa: E402
from concourse.bass_utils import run_bass_kernel_spmd  # noqa: E402

F16 = mybir.dt.float16
F32 = mybir.dt.float32
NPF16 = np.float16

FULL_CFG = dict(
    n_nodes=50000,
    n_cores=8,
    in_ch=128,
    out_ch=128,
    edge_dim=7,
    win=120,          # dst nodes per scatter window (8 lhsT rows for ea+bias)
    sentinel=500.0,   # dst_rel value for padded edges (no one-hot match)
)


# --------------------------------------------------------------------------
# host-side preparation: sort/shard/pad edges, build per-core input arrays
# --------------------------------------------------------------------------

def host_prep(x, edge_index, edge_attr, cfg):
    n_nodes = cfg["n_nodes"]
    n_cores = cfg["n_cores"]
    win = cfg["win"]
    ed = cfg["edge_dim"]
    npc = n_nodes // n_cores            # nodes per core
    n_win = -(-npc // win)              # windows per core

    src = np.asarray(edge_index[0], dtype=np.int64)
    dst = np.asarray(edge_index[1], dtype=np.int64)
    ea = np.asarray(edge_attr, dtype=np.float32)

    order = np.argsort(dst, kind="stable")
    src_s = src[order].astype(np.int64)
    dst_s = dst[order].astype(np.int32)
    ea_s = ea[order]

    deg_full = np.bincount(dst_s, minlength=n_nodes).astype(np.float32)
    x_f16 = np.asarray(x, np.float32).astype(NPF16)           # [N, ch]

    # per (core, window): edge ranges; TN = max tiles over cores
    core_bounds = np.searchsorted(dst_s, np.arange(n_cores + 1) * npc)
    wbounds = np.zeros((n_cores, n_win + 1), dtype=np.int64)
    TN = np.ones(n_win, dtype=np.int64)
    for c in range(n_cores):
        e0, e1 = core_bounds[c], core_bounds[c + 1]
        d_loc = dst_s[e0:e1] - c * npc
        wbounds[c] = e0 + np.searchsorted(d_loc, np.arange(n_win + 1) * win)
        for w in range(n_win):
            cnt = wbounds[c, w + 1] - wbounds[c, w]
            TN[w] = max(TN[w], -(-cnt // 128))
    T = int(TN.sum())
    off = np.zeros(n_win + 1, dtype=np.int64)
    off[1:] = np.cumsum(TN)

    in_maps = []
    for c in range(n_cores):
        src_arr = np.zeros(T * 128, dtype=np.int64)        # absolute src idx
        dstr = np.full(T * 128, cfg["sentinel"], dtype=np.float32)
        ea_arr = np.zeros((T * 128, ed), dtype=np.float32)
        for w in range(n_win):
            i0, i1 = wbounds[c, w], wbounds[c, w + 1]
            k = i1 - i0
            p0 = int(off[w]) * 128
            src_arr[p0:p0 + k] = src_s[i0:i1]
            dstr[p0:p0 + k] = (dst_s[i0:i1] - c * npc - w * win).astype(np.float32)
            ea_arr[p0:p0 + k] = ea_s[i0:i1]

        # merged lhsT per tile: [S_T(win); ea^T(ed); ones(1)] -> [128, T*128]
        dmat = dstr.reshape(T, 128)
        oneh = (dmat[:, :, None] == np.arange(win, dtype=np.float32)[None, None, :])
        st_m = np.zeros((128, T * 128), dtype=NPF16)
        st_m[:win] = oneh.transpose(0, 2, 1).reshape(T, win, 128).transpose(
            1, 0, 2).reshape(win, T * 128)
        st_m[win:win + ed] = ea_arr.T.astype(NPF16)
        st_m[127] = 1.0
        st_m = np.ascontiguousarray(st_m)

        # scatter rhs one-hot S per tile: [128 e, 128 cols] -> [128, T*128]
        s_pad = np.zeros((T, 128, 128), dtype=NPF16)
        s_pad[:, :, :win] = oneh
        s_oh = np.ascontiguousarray(s_pad.transpose(1, 0, 2).reshape(128, T * 128))

        # host-gathered x_j^T per tile: [128 ch, T*128 e]
        xjt = np.ascontiguousarray(x_f16[src_arr].T)

        xs = np.asarray(x[c * npc:(c + 1) * npc], dtype=np.float32)  # [npc, ch]
        x_t = np.ascontiguousarray(xs.T)                       # [ch, npc] f32
        x_16 = x_t.astype(NPF16)
        deg = deg_full[c * npc:(c + 1) * npc].reshape(1, npc).astype(NPF16)

        in_maps.append(dict(
            st_m=st_m, s_oh=s_oh, xjt=xjt,
            x_t=x_t, x_16=x_16, deg=deg,
        ))

    meta = dict(T=T, TN=TN, off=off, n_win=n_win, npc=npc)
    return in_maps, meta


def prep_weights(W1, b1, W2, b2, Wg, bg, W_ih, b_ih, W_hh, b_hh, gamma, beta, cfg):
    ic, oc, ed = cfg["in_ch"], cfg["out_ch"], cfg["edge_dim"]
    W1 = np.asarray(W1, np.float32)
    Wg = np.asarray(Wg, np.float32)
    # [W1e(7); b1(1)] rows for composite tiles
    web = np.zeros((8, oc), dtype=NPF16)
    web[:ed] = W1[2 * ic:2 * ic + ed].astype(NPF16)
    web[ed] = np.asarray(b1, np.float32).astype(NPF16)
    w = dict(
        W1i=np.ascontiguousarray(W1[0:ic]).astype(NPF16),
        Wj16=np.ascontiguousarray(W1[ic:2 * ic]).astype(NPF16),
        web=web,
        W2=np.asarray(W2, np.float32).astype(NPF16),
        Wac=(Wg[0:ic] + Wg[ic + oc:2 * ic + oc]).astype(NPF16),
        Wgb=Wg[ic:ic + oc].astype(NPF16),
        WihT=np.ascontiguousarray(np.asarray(W_ih, np.float32).T).astype(NPF16),
        WhhT=np.ascontiguousarray(np.asarray(W_hh, np.float32).T).astype(NPF16),
        b2r=np.asarray(b2, np.float32).reshape(1, oc).astype(NPF16),
        bgc=np.asarray(bg, np.float32).reshape(oc, 1),
        bihc=np.ascontiguousarray(np.asarray(b_ih, np.float32).reshape(3, ic).T),
        bhhc=np.ascontiguousarray(np.asarray(b_hh, np.float32).reshape(3, ic).T),
        gam=np.tile(np.asarray(gamma, np.float32).reshape(1, ic), (128, 1)),
        bet=np.tile(np.asarray(beta, np.float32).reshape(1, ic), (128, 1)),
    )
    return w


# --------------------------------------------------------------------------
# device program
# --------------------------------------------------------------------------

def build_program(cfg, meta):
    ic, oc, ed = cfg["in_ch"], cfg["out_ch"], cfg["edge_dim"]
    n_cores = cfg["n_cores"]
    win = cfg["win"]
    npc, n_win, T = meta["npc"], meta["n_win"], meta["T"]
    TN, off = meta["TN"], meta["off"]
    max_tiles = int(TN.max())

    nc = bacc.Bacc("TRN2", target_bir_lowering=False, debug=False,
                   num_devices=n_cores, num_swdge_queues=1)

    # ---- I/O ----
    stm_in = nc.dram_tensor("st_m", [128, T * 128], F16, kind="ExternalInput")
    s_in = nc.dram_tensor("s_oh", [128, T * 128], F16, kind="ExternalInput")
    xjt_in = nc.dram_tensor("xjt", [128, T * 128], F16, kind="ExternalInput")
    xt_in = nc.dram_tensor("x_t", [ic, npc], F32, kind="ExternalInput")
    x16_in = nc.dram_tensor("x_16", [ic, npc], F16, kind="ExternalInput")
    deg_in = nc.dram_tensor("deg", [1, npc], F16, kind="ExternalInput")
    w_in = {}
    wspecs = dict(W1i=([ic, oc], F16), Wj16=([ic, oc], F16), web=([8, oc], F16),
                  W2=([ic, oc], F16), Wac=([ic, oc], F16), Wgb=([oc, oc], F16),
                  WihT=([oc, 3 * ic], F16), WhhT=([ic, 3 * ic], F16),
                  b2r=([1, oc], F16), bgc=([oc, 1], F32),
                  bihc=([ic, 3], F32), bhhc=([ic, 3], F32),
                  gam=([128, ic], F32), bet=([128, ic], F32))
    for k, (shp, dt) in wspecs.items():
        w_in[k] = nc.dram_tensor(k, shp, dt, kind="ExternalInput")
    out_t = nc.dram_tensor("out", [npc, oc], F32, kind="ExternalOutput")

    with tile.TileContext(nc) as tc:
        with (
            tc.tile_pool(name="res", bufs=1) as res,       # resident tensors
            tc.tile_pool(name="psum", bufs=1, space="PSUM") as pp,
            tc.tile_pool(name="work", bufs=3) as wk,       # per-window work tiles
        ):
            # ---------- resident loads ----------
            xt_sb = res.tile([ic, npc], F32)
            nc.sync.dma_start(out=xt_sb[:], in_=xt_in[:])
            x16_sb = res.tile([ic, npc], F16)
            nc.sync.dma_start(out=x16_sb[:], in_=x16_in[:])
            deg_sb = res.tile([1, npc], F16)
            nc.sync.dma_start(out=deg_sb[:], in_=deg_in[:])
            w_sb = {}
            for k, (shp, dt) in wspecs.items():
                w_sb[k] = res.tile(shp, dt, tag=f"w_{k}", name=f"w_{k}")
                nc.scalar.dma_start(out=w_sb[k][:], in_=w_in[k][:])

            # ---------- constants ----------
            ident_f = res.tile([128, 128], F32)
            make_identity(nc, ident_f[:])
            ident_16 = res.tile([128, 128], F16)
            make_identity(nc, ident_16[:])
            eps_col = res.tile([128, 1], F32)
            nc.vector.memset(eps_col[:], 1e-5)
            bsum = res.tile([ic, 3], F32)              # b_ih + b_hh columns
            nc.vector.tensor_tensor(out=bsum[:], in0=w_sb["bihc"][:],
                                    in1=w_sb["bhhc"][:], op=mybir.AluOpType.add)
            zneg = res.tile([ic, 1], F32)              # -(b_ih_z + b_hh_z)
            nc.vector.tensor_scalar(out=zneg[:], in0=bsum[:, 1:2], scalar1=-1.0,
                                    scalar2=None, op0=mybir.AluOpType.mult)
            bihn2 = res.tile([ic, 1], F32)             # 2*b_ih_n (for tanh-as-sigmoid)
            nc.vector.tensor_scalar(out=bihn2[:], in0=w_sb["bihc"][:, 2:3],
                                    scalar1=2.0, scalar2=None,
                                    op0=mybir.AluOpType.mult)

            # aggregated messages (transposed), fp16 for GRU matmuls
            aggr_16 = res.tile([oc, npc], F16)

            # ---------- phase 0: P_i composites (just-in-time) ----------
            comps = {}

            def make_comp(w):
                n0 = w * win
                nj = min(win, npc - n0)
                comp = wk.tile([128, 128], F16, tag="comp", name=f"comp{w}",
                               bufs=4)
                if nj < win:
                    nc.vector.memset(comp[:], 0.0)
                ps_p = pp.tile([128, 128], F32, tag="AG", bufs=1)
                nc.tensor.matmul(out=ps_p[:nj, :oc], lhsT=x16_sb[:, n0:n0 + nj],
                                 rhs=w_sb["W1i"][:], start=True, stop=True,
                                 skip_group_check=True)
                nc.scalar.activation(out=comp[:nj, :oc], in_=ps_p[:nj, :oc],
                                     func=mybir.ActivationFunctionType.Copy)
                nc.scalar.dma_start(out=comp[win:, :], in_=w_in["web"][:])
                comps[w] = comp

            # ---------- node phase (512-wide compute, interleaved) ----------
            NB = 512
            n_nb = -(-npc // NB)

            def emit_node_block(j):
                n0 = j * NB
                nj = min(NB, npc - n0)
                xb = x16_sb[:, n0:n0 + nj]
                ab = aggr_16[:, n0:n0 + nj]
                xf = xt_sb[:, n0:n0 + nj]

                ps_r = pp.tile([128, NB], F32, tag="HPS", bufs=3, name="ps_r")
                nc.tensor.matmul(out=ps_r[:ic, :nj], lhsT=w_sb["WihT"][:, 0:ic],
                                 rhs=ab, start=True, stop=False, skip_group_check=True)
                nc.tensor.matmul(out=ps_r[:ic, :nj], lhsT=w_sb["WhhT"][:, 0:ic],
                                 rhs=xb, start=False, stop=True, skip_group_check=True)
                r_sb = wk.tile([128, NB], F32, tag="r_sb", name="r_sb")
                nc.scalar.activation(out=r_sb[:ic, :nj], in_=ps_r[:ic, :nj],
                                     func=mybir.ActivationFunctionType.Sigmoid,
                                     bias=bsum[:, 0:1])

                ps_z = pp.tile([128, NB], F32, tag="HPS", bufs=3, name="ps_z")
                nc.tensor.matmul(out=ps_z[:ic, :nj], lhsT=w_sb["WihT"][:, ic:2 * ic],
                                 rhs=ab, start=True, stop=False, skip_group_check=True)
                nc.tensor.matmul(out=ps_z[:ic, :nj], lhsT=w_sb["WhhT"][:, ic:2 * ic],
                                 rhs=xb, start=False, stop=True, skip_group_check=True)
                z_sb = wk.tile([128, NB], F32, tag="z_sb", name="z_sb")
                nc.scalar.activation(out=z_sb[:ic, :nj], in_=ps_z[:ic, :nj],
                                     func=mybir.ActivationFunctionType.Sigmoid,
                                     bias=bsum[:, 1:2])

                ps_gh = pp.tile([128, NB], F32, tag="AT", bufs=2, name="ps_gh")
                nc.tensor.matmul(out=ps_gh[:ic, :nj], lhsT=w_sb["WhhT"][:, 2 * ic:3 * ic],
                                 rhs=xb, start=True, stop=True, skip_group_check=True)
                hnb = wk.tile([128, NB], F32, tag="hnb", name="hnb")
                nc.scalar.activation(out=hnb[:ic, :nj], in_=ps_gh[:ic, :nj],
                                     func=mybir.ActivationFunctionType.Identity,
                                     bias=w_sb["bhhc"][:, 2:3])
                ps_gi = pp.tile([128, NB], F32, tag="AT", bufs=2, name="ps_gi")
                nc.tensor.matmul(out=ps_gi[:ic, :nj], lhsT=w_sb["WihT"][:, 2 * ic:3 * ic],
                                 rhs=ab, start=True, stop=True, skip_group_check=True)
                rgh = wk.tile([128, NB], F32, tag="rgh", name="rgh")
                nc.vector.tensor_tensor(out=rgh[:ic, :nj], in0=r_sb[:ic, :nj],
                                        in1=hnb[:ic, :nj], op=mybir.AluOpType.mult)
                npre = wk.tile([128, NB], F32, tag="npre", name="npre")
                nc.vector.tensor_tensor(out=npre[:ic, :nj], in0=ps_gi[:ic, :nj],
                                        in1=rgh[:ic, :nj], op=mybir.AluOpType.add)
                n_sb = wk.tile([128, NB], F32, tag="n_sb", name="n_sb")
                nc.scalar.activation(out=n_sb[:ic, :nj], in_=npre[:ic, :nj],
                                     func=mybir.ActivationFunctionType.Tanh,
                                     bias=w_sb["bihc"][:, 2:3])

                ps_g = pp.tile([128, NB], F32, tag="HPS", bufs=3, name="ps_g")
                nc.tensor.matmul(out=ps_g[:oc, :nj], lhsT=w_sb["Wac"][:], rhs=xb,
                                 start=True, stop=False, skip_group_check=True)
                nc.tensor.matmul(out=ps_g[:oc, :nj], lhsT=w_sb["Wgb"][:], rhs=ab,
                                 start=False, stop=True, skip_group_check=True)
                g_sb = wk.tile([128, NB], F32, tag="g_sb", name="g_sb")
                nc.scalar.activation(out=g_sb[:oc, :nj], in_=ps_g[:oc, :nj],
                                     func=mybir.ActivationFunctionType.Sigmoid,
                                     bias=w_sb["bgc"][:])

                # pre = x + g*(t1*z - t1), t1 = x - n
                t1 = wk.tile([128, NB], F32, tag="t1", name="t1")
                nc.vector.tensor_tensor(out=t1[:ic, :nj], in0=xf, in1=n_sb[:ic, :nj],
                                        op=mybir.AluOpType.subtract)
                u1 = wk.tile([128, NB], F32, tag="u1", name="u1")
                nc.vector.tensor_tensor(out=u1[:ic, :nj], in0=z_sb[:ic, :nj],
                                        in1=t1[:ic, :nj], op=mybir.AluOpType.mult)
                u2 = wk.tile([128, NB], F32, tag="u2", name="u2")
                nc.vector.tensor_tensor(out=u2[:ic, :nj], in0=u1[:ic, :nj],
                                        in1=t1[:ic, :nj], op=mybir.AluOpType.subtract)
                m2 = wk.tile([128, NB], F32, tag="m2", name="m2")
                nc.vector.tensor_tensor(out=m2[:ic, :nj], in0=g_sb[:oc, :nj],
                                        in1=u2[:ic, :nj], op=mybir.AluOpType.mult)
                pre16 = wk.tile([128, NB], F16, tag="pre16", name="pre16")
                nc.vector.tensor_tensor(out=pre16[:ic, :nj], in0=xf,
                                        in1=m2[:ic, :nj], op=mybir.AluOpType.add)

                # LN: transpose chunks, batched stats, normalize, store
                nch = -(-nj // 128)
                pst = []
                ssum = wk.tile([128, 4], F32, tag="ssum", name="ssum")
                qsum = wk.tile([128, 4], F32, tag="qsum", name="qsum")
                for hh in range(nch):
                    m0 = hh * 128
                    mj = min(128, nj - m0)
                    ps_t = pp.tile([128, 128], F16, tag="TR", bufs=2, name="ps_t")
                    nc.tensor.transpose(out=ps_t[:mj, :ic], in_=pre16[:ic, m0:m0 + mj],
                                        identity=ident_16[:])
                    sb_t = wk.tile([128, 128], F16, tag="sb_t", bufs=4, name="sb_t")
                    nc.scalar.activation(out=sb_t[:mj, :ic], in_=ps_t[:mj, :ic],
                                         func=mybir.ActivationFunctionType.Copy,
                                         accum_out=ssum[:mj, hh:hh + 1])
                    sqt = wk.tile([128, 128], F16, tag="sqt", name="sqt")
                    nc.scalar.activation(out=sqt[:mj, :ic], in_=sb_t[:mj, :ic],
                                         func=mybir.ActivationFunctionType.Square,
                                         accum_out=qsum[:mj, hh:hh + 1])
                    pst.append((sb_t, m0, mj))
                mu = wk.tile([128, 4], F32, tag="mu", name="mu")
                nc.vector.tensor_scalar(out=mu[:, :nch], in0=ssum[:, :nch],
                                        scalar1=1.0 / ic, scalar2=None,
                                        op0=mybir.AluOpType.mult)
                mu2 = wk.tile([128, 4], F32, tag="mu2", name="mu2")
                nc.vector.tensor_tensor(out=mu2[:, :nch], in0=mu[:, :nch],
                                        in1=mu[:, :nch], op=mybir.AluOpType.mult)
                qs2 = wk.tile([128, 4], F32, tag="qs2", name="qs2")
                nc.vector.tensor_scalar(out=qs2[:, :nch], in0=qsum[:, :nch],
                                        scalar1=1.0 / ic, scalar2=None,
                                        op0=mybir.AluOpType.mult)
                var = wk.tile([128, 4], F32, tag="var", name="var")
                nc.vector.tensor_tensor(out=var[:, :nch], in0=qs2[:, :nch],
                                        in1=mu2[:, :nch],
                                        op=mybir.AluOpType.subtract)
                sd = wk.tile([128, 4], F32, tag="sd", name="sd")
                nc.scalar.activation(out=sd[:, :nch], in_=var[:, :nch],
                                     func=mybir.ActivationFunctionType.Sqrt,
                                     bias=eps_col[:])
                rstd = wk.tile([128, 4], F32, tag="rstd", name="rstd")
                nc.vector.reciprocal(out=rstd[:, :nch], in_=sd[:, :nch])
                for hh, (sb_t, m0, mj) in enumerate(pst):
                    nrm = wk.tile([128, 128], F32, tag="nrm", name="nrm")
                    nc.vector.tensor_scalar(out=nrm[:mj, :ic], in0=sb_t[:mj, :ic],
                                            scalar1=mu[:mj, hh:hh + 1],
                                            scalar2=rstd[:mj, hh:hh + 1],
                                            op0=mybir.AluOpType.subtract,
                                            op1=mybir.AluOpType.mult)
                    sc = wk.tile([128, 128], F32, tag="sc", name="sc")
                    nc.vector.tensor_tensor(out=sc[:mj, :ic], in0=nrm[:mj, :ic],
                                            in1=w_sb["gam"][:mj, :ic],
                                            op=mybir.AluOpType.mult)
                    outf = wk.tile([128, 128], F32, tag="outf", name="outf")
                    nc.vector.tensor_tensor(out=outf[:mj, :ic], in0=sc[:mj, :ic],
                                            in1=w_sb["bet"][:mj, :ic],
                                            op=mybir.AluOpType.add)
                    nc.sync.dma_start(out=out_t[n0 + m0:n0 + m0 + mj, :],
                                      in_=outf[:mj, :ic])



            # ---------- edge phase ----------
            def load_stream(wnd, dram, tag, issuer):
                t0 = int(off[wnd])
                ntile = int(TN[wnd])
                tl = wk.tile([128, max_tiles * 128], F16, tag=tag)
                issuer.dma_start(out=tl[:, :ntile * 128],
                                 in_=dram[:, t0 * 128:(t0 + ntile) * 128])
                return tl

            def load_all(wnd):
                return (load_stream(wnd, stm_in, "stw", nc.sync),
                        load_stream(wnd, xjt_in, "xjw", nc.gpsimd),
                        load_stream(wnd, s_in, "sw", nc.scalar))

            pend = {}
            PREF = 2
            for wnd in range(min(PREF, n_win)):
                make_comp(wnd)
                pend[wnd] = load_all(wnd)
            next_nb = [0]

            for wnd in range(n_win):
                n0 = wnd * win
                nj = min(win, npc - n0)
                ntile = int(TN[wnd])
                stw, xjw, sw = pend.pop(wnd)
                if wnd + PREF < n_win:
                    make_comp(wnd + PREF)
                    pend[wnd + PREF] = load_all(wnd + PREF)

                at_ps = pp.tile([128, win], F32, tag="AT", bufs=2)

                groups = [(gs, min(4, ntile - gs)) for gs in range(0, ntile, 4)]
                prev = None   # (h_16, gstart, gn)

                def scatter_group(item, is_last_group):
                    h_16, gstart, gn = item
                    for t in range(gn):
                        tt = gstart + t
                        nc.tensor.matmul(
                            out=at_ps[:oc, :nj],
                            lhsT=h_16[:, t * 128:(t + 1) * 128],
                            rhs=sw[:, tt * 128:tt * 128 + nj],
                            start=(tt == 0), stop=is_last_group and (t == gn - 1),
                            skip_group_check=True)

                for gstart, gn in groups:
                    h_ps = pp.tile([128, 512], F32, tag="HPS", bufs=3)
                    for t in range(gn):
                        tt = gstart + t
                        nc.tensor.matmul(
                            out=h_ps[:, t * 128:(t + 1) * 128],
                            lhsT=stw[:, tt * 128:(tt + 1) * 128],
                            rhs=comps[wnd][:],
                            start=True, stop=False, skip_group_check=True)
                        nc.tensor.matmul(
                            out=h_ps[:, t * 128:(t + 1) * 128],
                            lhsT=xjw[:, tt * 128:(tt + 1) * 128],
                            rhs=w_sb["Wj16"][:],
                            start=False, stop=True, skip_group_check=True)
                    if prev is not None:
                        scatter_group(prev, False)
                    h_16 = wk.tile([128, 512], F16, tag="h16", bufs=3)
                    nc.scalar.activation(out=h_16[:, :gn * 128],
                                         in_=h_ps[:, :gn * 128],
                                         func=mybir.ActivationFunctionType.Relu)
                    prev = (h_16, gstart, gn)
                scatter_group(prev, True)

                # aggr_T = W2.T @ A_T + b2 (x) deg
                at_sb = wk.tile([128, win], F16, tag="at_sb")
                nc.vector.tensor_copy(out=at_sb[:oc, :nj], in_=at_ps[:oc, :nj])
                ps_ag = pp.tile([128, win], F32, tag="AG", bufs=1)
                nc.tensor.matmul(out=ps_ag[:oc, :nj], lhsT=w_sb["W2"][:],
                                 rhs=at_sb[:oc, :nj], start=True, stop=False,
                                 skip_group_check=True)
                nc.tensor.matmul(out=ps_ag[:oc, :nj], lhsT=w_sb["b2r"][:],
                                 rhs=deg_sb[:, n0:n0 + nj], start=False, stop=True,
                                 skip_group_check=True)
                nc.vector.tensor_copy(out=aggr_16[:, n0:n0 + nj], in_=ps_ag[:oc, :nj])

                coverage = npc if wnd == n_win - 1 else (wnd + 1) * win
                while (next_nb[0] * NB < npc
                       and ((next_nb[0] + 1) * NB <= coverage
                            or wnd == n_win - 1)):
                    emit_node_block(next_nb[0])
                    next_nb[0] += 1

    nc.compile()
    return nc


# --------------------------------------------------------------------------
# public entry
# --------------------------------------------------------------------------

_CACHE = {}


def kernel(x, edge_index, edge_attr, W1, b1, W2, b2, Wg, bg,
           W_ih, b_ih, W_hh, b_hh, gamma, beta, _cfg=None, _trace=None):
    if _trace is None:
        _trace = os.environ.get("GNN_TRACE", "0") == "1"
    cfg = dict(FULL_CFG if _cfg is None else _cfg)
    in_maps, meta = host_prep(x, edge_index, edge_attr, cfg)
    w = prep_weights(W1, b1, W2, b2, Wg, bg, W_ih, b_ih, W_hh, b_hh,
                     gamma, beta, cfg)
    for m in in_maps:
        m.update(w)

    key = (meta["T"], tuple(meta["TN"]))
    if key not in _CACHE:
        _CACHE.clear()
        _CACHE[key] = build_program(cfg, meta)
    nc = _CACHE[key]

    res = run_bass_kernel_spmd(nc, in_maps, list(range(cfg["n_cores"])),
                               trace=_trace)
    out = np.concatenate([res.results[c]["out"] for c in range(cfg["n_cores"])],
                         axis=0)
    kernel.last_results = res
    if _trace and res.exec_time_ns is not None:
        print(f"HW exec time: {res.exec_time_ns} ns")
        kernel.last_exec_time_ns = res.exec_time_ns
    return out.astype(np.float32)


# revision 24
# speedup vs baseline: 1.2430x; 1.2430x over previous
"""Bass/Trainium2 kernel for EnhancedGNNCap message passing (8 NeuronCores).

Strategy (node-sharded, edge-sorted, merged-lhsT edge phase, host-gathered x_j):
  - Sort edges by dst on host; shard nodes (and incoming edges) across 8
    cores; windows of 120 dst nodes; pad each window's edges to 128-edge
    tiles.
  - Host packs three [128, T*128] fp16 streams:
      st_m: per tile lhsT = [S_T(120); ea^T(7); ones(1)]  (merged gather)
      xjt:  per tile lhsT = x[src_e]^T                    (host-gathered)
      s_oh: per tile rhs  = one-hot S [e, node]           (scatter)
  - Phase 0 (device): P_i composite tiles [P_i_win; W1e; b1] (f32 matmul,
    fp16 store).
  - Edge phase per tile: h_ps = st_m.T @ comp + xjt.T @ W1j (PSUM
    accumulate), ReLU (4 tiles batched) -> fp16, scatter-accumulate
    A_T += h.T @ S.
  - Window close: aggr_T = W2.T @ A_T + b2 (x) deg  (f32).
  - Node phase: GRU + gate + LayerNorm in [ch, node] orientation,
    transpose, write out.
All per-core differences are carried in input data; one SPMD program.
"""

import os
import sys
import types

sys.path.insert(0, "/opt/trn_rl_repo")

import numpy as np


def _install_ntff_hook():
    """Register the axon NTFF profiling hook if the image lacks antenv.axon_hooks."""
    try:
        import antenv
        try:
            import antenv.axon_hooks  # noqa: F401
            return
        except ImportError:
            pass
        m = types.ModuleType("antenv.axon_hooks")
        m._hook = None
        m.set_axon_ntff_profile_hook = lambda h: setattr(m, "_hook", h)
        m.get_axon_ntff_profile_hook = lambda: m._hook
        sys.modules["antenv.axon_hooks"] = m
        antenv.axon_hooks = m
        from trn_agent_boot.trn_boot import _ntff_profile_via_ctypes
        m.set_axon_ntff_profile_hook(_ntff_profile_via_ctypes("/opt/axon/libaxon_pjrt.so"))
    except Exception:
        pass


_install_ntff_hook()

import concourse.bass as bass  # noqa: E402
import concourse.bacc as bacc  # noqa: E402
import concourse.mybir as mybir  # noqa: E402
import concourse.tile as tile  # noqa: E402
from concourse.masks import make_identity  # noqa: E402
from concourse.bass_utils import run_bass_kernel_spmd  # noqa: E402

F16 = mybir.dt.float16
F32 = mybir.dt.float32
NPF16 = np.float16

FULL_CFG = dict(
    n_nodes=50000,
    n_cores=8,
    in_ch=128,
    out_ch=128,
    edge_dim=7,
    win=120,          # dst nodes per scatter window (8 lhsT rows for ea+bias)
    sentinel=500.0,   # dst_rel value for padded edges (no one-hot match)
)


# --------------------------------------------------------------------------
# host-side preparation: sort/shard/pad edges, build per-core input arrays
# --------------------------------------------------------------------------

def host_prep(x, edge_index, edge_attr, cfg):
    n_nodes = cfg["n_nodes"]
    n_cores = cfg["n_cores"]
    win = cfg["win"]
    ed = cfg["edge_dim"]
    npc = n_nodes // n_cores            # nodes per core
    n_win = -(-npc // win)              # windows per core

    src = np.asarray(edge_index[0], dtype=np.int64)
    dst = np.asarray(edge_index[1], dtype=np.int64)
    ea = np.asarray(edge_attr, dtype=np.float32)

    order = np.argsort(dst, kind="stable")
    src_s = src[order].astype(np.int64)
    dst_s = dst[order].astype(np.int32)
    ea_s = ea[order]

    deg_full = np.bincount(dst_s, minlength=n_nodes).astype(np.float32)
    x_f16 = np.asarray(x, np.float32).astype(NPF16)           # [N, ch]

    # per (core, window): edge ranges; TN = max tiles over cores
    core_bounds = np.searchsorted(dst_s, np.arange(n_cores + 1) * npc)
    wbounds = np.zeros((n_cores, n_win + 1), dtype=np.int64)
    TN = np.ones(n_win, dtype=np.int64)
    for c in range(n_cores):
        e0, e1 = core_bounds[c], core_bounds[c + 1]
        d_loc = dst_s[e0:e1] - c * npc
        wbounds[c] = e0 + np.searchsorted(d_loc, np.arange(n_win + 1) * win)
        for w in range(n_win):
            cnt = wbounds[c, w + 1] - wbounds[c, w]
            TN[w] = max(TN[w], -(-cnt // 128))
    T = int(TN.sum())
    off = np.zeros(n_win + 1, dtype=np.int64)
    off[1:] = np.cumsum(TN)

    in_maps = []
    for c in range(n_cores):
        src_arr = np.zeros(T * 128, dtype=np.int64)        # absolute src idx
        dstr = np.full(T * 128, cfg["sentinel"], dtype=np.float32)
        ea_arr = np.zeros((T * 128, ed), dtype=np.float32)
        for w in range(n_win):
            i0, i1 = wbounds[c, w], wbounds[c, w + 1]
            k = i1 - i0
            p0 = int(off[w]) * 128
            src_arr[p0:p0 + k] = src_s[i0:i1]
            dstr[p0:p0 + k] = (dst_s[i0:i1] - c * npc - w * win).astype(np.float32)
            ea_arr[p0:p0 + k] = ea_s[i0:i1]

        # merged lhsT per tile: [S_T(win); ea^T(ed); ones(1)] -> [128, T*128]
        dmat = dstr.reshape(T, 128)
        oneh = (dmat[:, :, None] == np.arange(win, dtype=np.float32)[None, None, :])
        st_m = np.zeros((128, T * 128), dtype=NPF16)
        st_m[:win] = oneh.transpose(0, 2, 1).reshape(T, win, 128).transpose(
            1, 0, 2).reshape(win, T * 128)
        st_m[win:win + ed] = ea_arr.T.astype(NPF16)
        st_m[127] = 1.0
        st_m = np.ascontiguousarray(st_m)

        # scatter rhs one-hot S per tile: [128 e, 128 cols] -> [128, T*128]
        s_pad = np.zeros((T, 128, 128), dtype=NPF16)
        s_pad[:, :, :win] = oneh
        s_oh = np.ascontiguousarray(s_pad.transpose(1, 0, 2).reshape(128, T * 128))

        # host-gathered x_j^T per tile: [128 ch, T*128 e]
        xjt = np.ascontiguousarray(x_f16[src_arr].T)

        xs = np.asarray(x[c * npc:(c + 1) * npc], dtype=np.float32)  # [npc, ch]
        x_t = np.ascontiguousarray(xs.T)                       # [ch, npc] f32
        x_16 = x_t.astype(NPF16)
        deg = deg_full[c * npc:(c + 1) * npc].reshape(1, npc).astype(NPF16)

        in_maps.append(dict(
            st_m=st_m, s_oh=s_oh, xjt=xjt,
            x_t=x_t, x_16=x_16, deg=deg,
        ))

    meta = dict(T=T, TN=TN, off=off, n_win=n_win, npc=npc)
    return in_maps, meta


def prep_weights(W1, b1, W2, b2, Wg, bg, W_ih, b_ih, W_hh, b_hh, gamma, beta, cfg):
    ic, oc, ed = cfg["in_ch"], cfg["out_ch"], cfg["edge_dim"]
    W1 = np.asarray(W1, np.float32)
    Wg = np.asarray(Wg, np.float32)
    # [W1e(7); b1(1)] rows for composite tiles
    web = np.zeros((8, oc), dtype=NPF16)
    web[:ed] = W1[2 * ic:2 * ic + ed].astype(NPF16)
    web[ed] = np.asarray(b1, np.float32).astype(NPF16)
    w = dict(
        W1i=np.ascontiguousarray(W1[0:ic]).astype(NPF16),
        Wj16=np.ascontiguousarray(W1[ic:2 * ic]).astype(NPF16),
        web=web,
        W2=np.asarray(W2, np.float32).astype(NPF16),
        Wac=(Wg[0:ic] + Wg[ic + oc:2 * ic + oc]).astype(NPF16),
        Wgb=Wg[ic:ic + oc].astype(NPF16),
        WihT=np.ascontiguousarray(np.asarray(W_ih, np.float32).T).astype(NPF16),
        WhhT=np.ascontiguousarray(np.asarray(W_hh, np.float32).T).astype(NPF16),
        b2r=np.asarray(b2, np.float32).reshape(1, oc).astype(NPF16),
        bgc=np.asarray(bg, np.float32).reshape(oc, 1),
        bihc=np.ascontiguousarray(np.asarray(b_ih, np.float32).reshape(3, ic).T),
        bhhc=np.ascontiguousarray(np.asarray(b_hh, np.float32).reshape(3, ic).T),
        gam=np.tile(np.asarray(gamma, np.float32).reshape(1, ic), (128, 1)),
        bet=np.tile(np.asarray(beta, np.float32).reshape(1, ic), (128, 1)),
    )
    return w


# --------------------------------------------------------------------------
# device program
# --------------------------------------------------------------------------

def build_program(cfg, meta):
    ic, oc, ed = cfg["in_ch"], cfg["out_ch"], cfg["edge_dim"]
    n_cores = cfg["n_cores"]
    win = cfg["win"]
    npc, n_win, T = meta["npc"], meta["n_win"], meta["T"]
    TN, off = meta["TN"], meta["off"]
    max_tiles = int(TN.max())

    nc = bacc.Bacc("TRN2", target_bir_lowering=False, debug=False,
                   num_devices=n_cores, num_swdge_queues=1)

    # ---- I/O ----
    stm_in = nc.dram_tensor("st_m", [128, T * 128], F16, kind="ExternalInput")
    s_in = nc.dram_tensor("s_oh", [128, T * 128], F16, kind="ExternalInput")
    xjt_in = nc.dram_tensor("xjt", [128, T * 128], F16, kind="ExternalInput")
    xt_in = nc.dram_tensor("x_t", [ic, npc], F32, kind="ExternalInput")
    x16_in = nc.dram_tensor("x_16", [ic, npc], F16, kind="ExternalInput")
    deg_in = nc.dram_tensor("deg", [1, npc], F16, kind="ExternalInput")
    w_in = {}
    wspecs = dict(W1i=([ic, oc], F16), Wj16=([ic, oc], F16), web=([8, oc], F16),
                  W2=([ic, oc], F16), Wac=([ic, oc], F16), Wgb=([oc, oc], F16),
                  WihT=([oc, 3 * ic], F16), WhhT=([ic, 3 * ic], F16),
                  b2r=([1, oc], F16), bgc=([oc, 1], F32),
                  bihc=([ic, 3], F32), bhhc=([ic, 3], F32),
                  gam=([128, ic], F32), bet=([128, ic], F32))
    for k, (shp, dt) in wspecs.items():
        w_in[k] = nc.dram_tensor(k, shp, dt, kind="ExternalInput")
    out_t = nc.dram_tensor("out", [npc, oc], F32, kind="ExternalOutput")

    with tile.TileContext(nc) as tc:
        with (
            tc.tile_pool(name="res", bufs=1) as res,       # resident tensors
            tc.tile_pool(name="psum", bufs=1, space="PSUM") as pp,
            tc.tile_pool(name="work", bufs=3) as wk,       # per-window work tiles
        ):
            # ---------- resident loads ----------
            xt_sb = res.tile([ic, npc], F32)
            nc.sync.dma_start(out=xt_sb[:], in_=xt_in[:])
            x16_sb = res.tile([ic, npc], F16)
            nc.sync.dma_start(out=x16_sb[:], in_=x16_in[:])
            deg_sb = res.tile([1, npc], F16)
            nc.sync.dma_start(out=deg_sb[:], in_=deg_in[:])
            w_sb = {}
            for k, (shp, dt) in wspecs.items():
                w_sb[k] = res.tile(shp, dt, tag=f"w_{k}", name=f"w_{k}")
                nc.scalar.dma_start(out=w_sb[k][:], in_=w_in[k][:])

            # ---------- constants ----------
            ident_f = res.tile([128, 128], F32)
            make_identity(nc, ident_f[:])
            ident_16 = res.tile([128, 128], F16)
            make_identity(nc, ident_16[:])
            eps_col = res.tile([128, 1], F32)
            nc.vector.memset(eps_col[:], 1e-5)
            bsum = res.tile([ic, 3], F32)              # b_ih + b_hh columns
            nc.vector.tensor_tensor(out=bsum[:], in0=w_sb["bihc"][:],
                                    in1=w_sb["bhhc"][:], op=mybir.AluOpType.add)
            zneg = res.tile([ic, 1], F32)              # -(b_ih_z + b_hh_z)
            nc.vector.tensor_scalar(out=zneg[:], in0=bsum[:, 1:2], scalar1=-1.0,
                                    scalar2=None, op0=mybir.AluOpType.mult)
            bihn2 = res.tile([ic, 1], F32)             # 2*b_ih_n (for tanh-as-sigmoid)
            nc.vector.tensor_scalar(out=bihn2[:], in0=w_sb["bihc"][:, 2:3],
                                    scalar1=2.0, scalar2=None,
                                    op0=mybir.AluOpType.mult)

            # aggregated messages (transposed), fp16 for GRU matmuls
            aggr_16 = res.tile([oc, npc], F16)

            # ---------- phase 0: P_i composites (just-in-time) ----------
            comps = {}

            def make_comp(w):
                n0 = w * win
                nj = min(win, npc - n0)
                comp = wk.tile([128, 128], F16, tag="comp", name=f"comp{w}",
                               bufs=4)
                if nj < win:
                    nc.vector.memset(comp[:], 0.0)
                ps_p = pp.tile([128, 128], F32, tag="AG", bufs=1)
                nc.tensor.matmul(out=ps_p[:nj, :oc], lhsT=x16_sb[:, n0:n0 + nj],
                                 rhs=w_sb["W1i"][:], start=True, stop=True,
                                 skip_group_check=True)
                nc.scalar.activation(out=comp[:nj, :oc], in_=ps_p[:nj, :oc],
                                     func=mybir.ActivationFunctionType.Copy)
                nc.scalar.dma_start(out=comp[win:, :], in_=w_in["web"][:])
                comps[w] = comp

            # ---------- node phase (512-wide compute, interleaved) ----------
            NB = 512
            n_nb = -(-npc // NB)

            def emit_node_block(j):
                n0 = j * NB
                nj = min(NB, npc - n0)
                xb = x16_sb[:, n0:n0 + nj]
                ab = aggr_16[:, n0:n0 + nj]
                xf = xt_sb[:, n0:n0 + nj]

                ps_r = pp.tile([128, NB], F32, tag="HPS", bufs=3, name="ps_r")
                nc.tensor.matmul(out=ps_r[:ic, :nj], lhsT=w_sb["WihT"][:, 0:ic],
                                 rhs=ab, start=True, stop=False, skip_group_check=True)
                nc.tensor.matmul(out=ps_r[:ic, :nj], lhsT=w_sb["WhhT"][:, 0:ic],
                                 rhs=xb, start=False, stop=True, skip_group_check=True)
                r_sb = wk.tile([128, NB], F32, tag="r_sb", name="r_sb")
                nc.scalar.activation(out=r_sb[:ic, :nj], in_=ps_r[:ic, :nj],
                                     func=mybir.ActivationFunctionType.Sigmoid,
                                     bias=bsum[:, 0:1])

                ps_z = pp.tile([128, NB], F32, tag="HPS", bufs=3, name="ps_z")
                nc.tensor.matmul(out=ps_z[:ic, :nj], lhsT=w_sb["WihT"][:, ic:2 * ic],
                                 rhs=ab, start=True, stop=False, skip_group_check=True)
                nc.tensor.matmul(out=ps_z[:ic, :nj], lhsT=w_sb["WhhT"][:, ic:2 * ic],
                                 rhs=xb, start=False, stop=True, skip_group_check=True)
                z_sb = wk.tile([128, NB], F32, tag="z_sb", name="z_sb")
                nc.scalar.activation(out=z_sb[:ic, :nj], in_=ps_z[:ic, :nj],
                                     func=mybir.ActivationFunctionType.Sigmoid,
                                     bias=bsum[:, 1:2])

                ps_gh = pp.tile([128, NB], F32, tag="AT", bufs=2, name="ps_gh")
                nc.tensor.matmul(out=ps_gh[:ic, :nj], lhsT=w_sb["WhhT"][:, 2 * ic:3 * ic],
                                 rhs=xb, start=True, stop=True, skip_group_check=True)
                hnb = wk.tile([128, NB], F32, tag="hnb", name="hnb")
                nc.scalar.activation(out=hnb[:ic, :nj], in_=ps_gh[:ic, :nj],
                                     func=mybir.ActivationFunctionType.Identity,
                                     bias=w_sb["bhhc"][:, 2:3])
                ps_gi = pp.tile([128, NB], F32, tag="AT", bufs=2, name="ps_gi")
                nc.tensor.matmul(out=ps_gi[:ic, :nj], lhsT=w_sb["WihT"][:, 2 * ic:3 * ic],
                                 rhs=ab, start=True, stop=True, skip_group_check=True)
                rgh = wk.tile([128, NB], F32, tag="rgh", name="rgh")
                nc.vector.tensor_tensor(out=rgh[:ic, :nj], in0=r_sb[:ic, :nj],
                                        in1=hnb[:ic, :nj], op=mybir.AluOpType.mult)
                npre = wk.tile([128, NB], F32, tag="npre", name="npre")
                nc.vector.tensor_tensor(out=npre[:ic, :nj], in0=ps_gi[:ic, :nj],
                                        in1=rgh[:ic, :nj], op=mybir.AluOpType.add)
                n_sb = wk.tile([128, NB], F32, tag="n_sb", name="n_sb")
                nc.scalar.activation(out=n_sb[:ic, :nj], in_=npre[:ic, :nj],
                                     func=mybir.ActivationFunctionType.Tanh,
                                     bias=w_sb["bihc"][:, 2:3])

                ps_g = pp.tile([128, NB], F32, tag="HPS", bufs=3, name="ps_g")
                nc.tensor.matmul(out=ps_g[:oc, :nj], lhsT=w_sb["Wac"][:], rhs=xb,
                                 start=True, stop=False, skip_group_check=True)
                nc.tensor.matmul(out=ps_g[:oc, :nj], lhsT=w_sb["Wgb"][:], rhs=ab,
                                 start=False, stop=True, skip_group_check=True)
                g_sb = wk.tile([128, NB], F32, tag="g_sb", name="g_sb")
                nc.scalar.activation(out=g_sb[:oc, :nj], in_=ps_g[:oc, :nj],
                                     func=mybir.ActivationFunctionType.Sigmoid,
                                     bias=w_sb["bgc"][:])

                # pre = x + g*(t1*z - t1), t1 = x - n
                t1 = wk.tile([128, NB], F32, tag="t1", name="t1")
                nc.vector.tensor_tensor(out=t1[:ic, :nj], in0=xf, in1=n_sb[:ic, :nj],
                                        op=mybir.AluOpType.subtract)
                u1 = wk.tile([128, NB], F32, tag="u1", name="u1")
                nc.vector.tensor_tensor(out=u1[:ic, :nj], in0=z_sb[:ic, :nj],
                                        in1=t1[:ic, :nj], op=mybir.AluOpType.mult)
                u2 = wk.tile([128, NB], F32, tag="u2", name="u2")
                nc.vector.tensor_tensor(out=u2[:ic, :nj], in0=u1[:ic, :nj],
                                        in1=t1[:ic, :nj], op=mybir.AluOpType.subtract)
                m2 = wk.tile([128, NB], F32, tag="m2", name="m2")
                nc.vector.tensor_tensor(out=m2[:ic, :nj], in0=g_sb[:oc, :nj],
                                        in1=u2[:ic, :nj], op=mybir.AluOpType.mult)
                pre16 = wk.tile([128, NB], F16, tag="pre16", name="pre16")
                nc.vector.tensor_tensor(out=pre16[:ic, :nj], in0=xf,
                                        in1=m2[:ic, :nj], op=mybir.AluOpType.add)

                # LN: transpose chunks, batched stats, normalize, store
                nch = -(-nj // 128)
                pst = []
                ssum = wk.tile([128, 4], F32, tag="ssum", name="ssum")
                qsum = wk.tile([128, 4], F32, tag="qsum", name="qsum")
                for hh in range(nch):
                    m0 = hh * 128
                    mj = min(128, nj - m0)
                    ps_t = pp.tile([128, 128], F16, tag="TR", bufs=2, name="ps_t")
                    nc.tensor.transpose(out=ps_t[:mj, :ic], in_=pre16[:ic, m0:m0 + mj],
                                        identity=ident_16[:])
                    sb_t = wk.tile([128, 128], F16, tag="sb_t", bufs=4, name="sb_t")
                    nc.scalar.activation(out=sb_t[:mj, :ic], in_=ps_t[:mj, :ic],
                                         func=mybir.ActivationFunctionType.Copy,
                                         accum_out=ssum[:mj, hh:hh + 1])
                    sqt = wk.tile([128, 128], F16, tag="sqt", name="sqt")
                    nc.scalar.activation(out=sqt[:mj, :ic], in_=sb_t[:mj, :ic],
                                         func=mybir.ActivationFunctionType.Square,
                                         accum_out=qsum[:mj, hh:hh + 1])
                    pst.append((sb_t, m0, mj))
                mu = wk.tile([128, 4], F32, tag="mu", name="mu")
                nc.vector.tensor_scalar(out=mu[:, :nch], in0=ssum[:, :nch],
                                        scalar1=1.0 / ic, scalar2=None,
                                        op0=mybir.AluOpType.mult)
                mu2 = wk.tile([128, 4], F32, tag="mu2", name="mu2")
                nc.vector.tensor_tensor(out=mu2[:, :nch], in0=mu[:, :nch],
                                        in1=mu[:, :nch], op=mybir.AluOpType.mult)
                qs2 = wk.tile([128, 4], F32, tag="qs2", name="qs2")
                nc.vector.tensor_scalar(out=qs2[:, :nch], in0=qsum[:, :nch],
                                        scalar1=1.0 / ic, scalar2=None,
                                        op0=mybir.AluOpType.mult)
                var = wk.tile([128, 4], F32, tag="var", name="var")
                nc.vector.tensor_tensor(out=var[:, :nch], in0=qs2[:, :nch],
                                        in1=mu2[:, :nch],
                                        op=mybir.AluOpType.subtract)
                sd = wk.tile([128, 4], F32, tag="sd", name="sd")
                nc.scalar.activation(out=sd[:, :nch], in_=var[:, :nch],
                                     func=mybir.ActivationFunctionType.Sqrt,
                                     bias=eps_col[:])
                rstd = wk.tile([128, 4], F32, tag="rstd", name="rstd")
                nc.vector.reciprocal(out=rstd[:, :nch], in_=sd[:, :nch])
                for hh, (sb_t, m0, mj) in enumerate(pst):
                    nrm = wk.tile([128, 128], F32, tag="nrm", name="nrm")
                    nc.vector.tensor_scalar(out=nrm[:mj, :ic], in0=sb_t[:mj, :ic],
                                            scalar1=mu[:mj, hh:hh + 1],
                                            scalar2=rstd[:mj, hh:hh + 1],
                                            op0=mybir.AluOpType.subtract,
                                            op1=mybir.AluOpType.mult)
                    sc = wk.tile([128, 128], F32, tag="sc", name="sc")
                    nc.vector.tensor_tensor(out=sc[:mj, :ic], in0=nrm[:mj, :ic],
                                            in1=w_sb["gam"][:mj, :ic],
                                            op=mybir.AluOpType.mult)
                    outf = wk.tile([128, 128], F32, tag="outf", name="outf")
                    nc.vector.tensor_tensor(out=outf[:mj, :ic], in0=sc[:mj, :ic],
                                            in1=w_sb["bet"][:mj, :ic],
                                            op=mybir.AluOpType.add)
                    nc.sync.dma_start(out=out_t[n0 + m0:n0 + m0 + mj, :],
                                      in_=outf[:mj, :ic])



            # ---------- edge phase ----------
            def load_stream(wnd, dram, tag, issuer):
                t0 = int(off[wnd])
                ntile = int(TN[wnd])
                tl = wk.tile([128, max_tiles * 128], F16, tag=tag)
                issuer.dma_start(out=tl[:, :ntile * 128],
                                 in_=dram[:, t0 * 128:(t0 + ntile) * 128])
                return tl

            def load_all(wnd):
                return (load_stream(wnd, stm_in, "stw", nc.sync),
                        load_stream(wnd, xjt_in, "xjw", nc.gpsimd),
                        load_stream(wnd, s_in, "sw", nc.scalar))

            pend = {}
            PREF = 2
            for wnd in range(min(PREF, n_win)):
                make_comp(wnd)
                pend[wnd] = load_all(wnd)
            next_nb = [0]

            for wnd in range(n_win):
                n0 = wnd * win
                nj = min(win, npc - n0)
                ntile = int(TN[wnd])
                stw, xjw, sw = pend.pop(wnd)
                if wnd + PREF < n_win:
                    make_comp(wnd + PREF)
                    pend[wnd + PREF] = load_all(wnd + PREF)

                at_ps = pp.tile([128, win], F32, tag="AT", bufs=2)

                groups = [(gs, min(4, ntile - gs)) for gs in range(0, ntile, 4)]
                prev = None   # (h_16, gstart, gn)

                def scatter_group(item, is_last_group):
                    h_16, gstart, gn = item
                    for t in range(gn):
                        tt = gstart + t
                        nc.tensor.matmul(
                            out=at_ps[:oc, :nj],
                            lhsT=h_16[:, t * 128:(t + 1) * 128],
                            rhs=sw[:, tt * 128:tt * 128 + nj],
                            start=(tt == 0), stop=is_last_group and (t == gn - 1),
                            skip_group_check=True)

                for gstart, gn in groups:
                    h_ps = pp.tile([128, 512], F32, tag="HPS", bufs=3)
                    for t in range(gn):
                        tt = gstart + t
                        nc.tensor.matmul(
                            out=h_ps[:, t * 128:(t + 1) * 128],
                            lhsT=stw[:, tt * 128:(tt + 1) * 128],
                            rhs=comps[wnd][:],
                            start=True, stop=False, skip_group_check=True)
                        nc.tensor.matmul(
                            out=h_ps[:, t * 128:(t + 1) * 128],
                            lhsT=xjw[:, tt * 128:(tt + 1) * 128],
                            rhs=w_sb["Wj16"][:],
                            start=False, stop=True, skip_group_check=True)
                    if prev is not None:
                        scatter_group(prev, False)
                    h_16 = wk.tile([128, 512], F16, tag="h16", bufs=3)
                    nc.vector.tensor_scalar(out=h_16[:, :gn * 128],
                                            in0=h_ps[:, :gn * 128],
                                            scalar1=0.0, scalar2=None,
                                            op0=mybir.AluOpType.max)
                    prev = (h_16, gstart, gn)
                scatter_group(prev, True)

                # aggr_T = W2.T @ A_T + b2 (x) deg
                at_sb = wk.tile([128, win], F16, tag="at_sb")
                nc.vector.tensor_copy(out=at_sb[:oc, :nj], in_=at_ps[:oc, :nj])
                ps_ag = pp.tile([128, win], F32, tag="AG", bufs=1)
                nc.tensor.matmul(out=ps_ag[:oc, :nj], lhsT=w_sb["W2"][:],
                                 rhs=at_sb[:oc, :nj], start=True, stop=False,
                                 skip_group_check=True)
                nc.tensor.matmul(out=ps_ag[:oc, :nj], lhsT=w_sb["b2r"][:],
                                 rhs=deg_sb[:, n0:n0 + nj], start=False, stop=True,
                                 skip_group_check=True)
                nc.vector.tensor_copy(out=aggr_16[:, n0:n0 + nj], in_=ps_ag[:oc, :nj])

                coverage = npc if wnd == n_win - 1 else (wnd + 1) * win
                while (next_nb[0] * NB < npc
                       and ((next_nb[0] + 1) * NB <= coverage
                            or wnd == n_win - 1)):
                    emit_node_block(next_nb[0])
                    next_nb[0] += 1

    nc.compile()
    return nc


# --------------------------------------------------------------------------
# public entry
# --------------------------------------------------------------------------

_CACHE = {}


def kernel(x, edge_index, edge_attr, W1, b1, W2, b2, Wg, bg,
           W_ih, b_ih, W_hh, b_hh, gamma, beta, _cfg=None, _trace=None):
    if _trace is None:
        _trace = os.environ.get("GNN_TRACE", "0") == "1"
    cfg = dict(FULL_CFG if _cfg is None else _cfg)
    in_maps, meta = host_prep(x, edge_index, edge_attr, cfg)
    w = prep_weights(W1, b1, W2, b2, Wg, bg, W_ih, b_ih, W_hh, b_hh,
                     gamma, beta, cfg)
    for m in in_maps:
        m.update(w)

    key = (meta["T"], tuple(meta["TN"]))
    if key not in _CACHE:
        _CACHE.clear()
        _CACHE[key] = build_program(cfg, meta)
    nc = _CACHE[key]

    res = run_bass_kernel_spmd(nc, in_maps, list(range(cfg["n_cores"])),
                               trace=_trace)
    out = np.concatenate([res.results[c]["out"] for c in range(cfg["n_cores"])],
                         axis=0)
    kernel.last_results = res
    if _trace and res.exec_time_ns is not None:
        print(f"HW exec time: {res.exec_time_ns} ns")
        kernel.last_exec_time_ns = res.exec_time_ns
    return out.astype(np.float32)
